# revision 53
# baseline (speedup 1.0000x reference)
"""ColourCatGINConv on 8 TRN2 NeuronCores.

Strategy: shard GIN aggregation by destination-node range (12500 nodes/core).
Each core gathers x[src] rows for its incoming edges via SWDGE dma_gather
(int16 indices -> the 100k-row table is split into 4 quadrants of 25000 rows),
and segment-sums them with a one-hot matmul on the TensorEngine:

    P_x[f, d] = sum_e x[src_e][f] * onehot[e, d]     (lhsT = gathered rows)

Gather slots are packed contiguously per (superchunk, quadrant) instruction
per core — no interior padding. Tiles may straddle dst blocks and block
boundaries differ per core; each block's one-hot covers the compile-time
UNION tile range over all cores, with foreign/pad slots masked to a sentinel
in the per-core dstloc table. Trailing pad indices are negative so the SWDGE
ucode truncates them before descriptor generation (the Q7 descriptor rate,
~8ns/idx, is the kernel bottleneck).

The colour channel never touches the device gather: with G = [x, c, 1],
y1 = Z @ [[W1_x], [colour_W @ W1_c], [colour_b @ W1_c]] and the [c, 1]
aggregates (per-dst colour sum and degree) are index metadata computed on
host and shipped as a z-ready [2, NPAD] input. BN batch stats are
partial-summed per core and AllReduced.
"""

import os
import numpy as np
import ml_dtypes

NO_GATHER = bool(int(os.environ.get("KERNEL_NO_GATHER", "0")))
OH_MODE = os.environ.get("KERNEL_OH", "dma")   # dma | split
BF16 = ml_dtypes.bfloat16
FP8 = ml_dtypes.float8_e4m3
P = 128
NC = 8
NQ = 4
SC = 6          # dst blocks (128 nodes each) per superchunk
SENT = 255.0    # dstloc sentinel for foreign/pad slots


# ----------------------------------------------------------------- host prep

def _prep(x, c, edge_index, colour_W, colour_b, eps, W1, gamma, beta, W2, b2):
    N, D = x.shape
    NPC = N // NC
    NBLK = (NPC + P - 1) // P
    NPAD = NBLK * P
    QROWS = N // NQ
    NSC = (NBLK + SC - 1) // SC
    opeps = float(1.0 + np.asarray(eps, np.float32).reshape(-1)[0])

    x = np.ascontiguousarray(x, np.float32)
    cflat = np.asarray(c, np.float32).reshape(-1)
    src = np.asarray(edge_index[0], np.int64)
    dst = np.asarray(edge_index[1], np.int64)

    core = dst // NPC
    b_all = (dst % NPC) // P
    q_all = src // QROWS
    # gather instructions span GSC superchunks (coarser granularity cuts the
    # per-instruction ceil padding and fixed cost); PSUM epilogues still
    # process SC blocks at a time.
    GSC = 1
    GB = GSC * SC
    NSP = (NBLK + GB - 1) // GB
    s_all = b_all // GB
    sblocks = [list(range(s * GB, min((s + 1) * GB, NBLK))) for s in range(NSP)]
    NSC = NSP

    # per-core per-(s,q,b) counts -> cumulative offsets within each (s,q)
    grp_b = ((core * NSC + s_all) * NQ + q_all) * NBLK + b_all
    cnt = np.bincount(grp_b, minlength=NC * NSC * NQ * NBLK)
    cnt = cnt.reshape(NC, NSC, NQ, NBLK)

    # per-core block start offsets within (s,q); per-core (s,q) totals
    off_b = np.zeros_like(cnt)
    tot_sq = np.zeros((NC, NSC, NQ), np.int64)
    for s in range(NSC):
        blks = sblocks[s]
        run = np.zeros((NC, NQ), np.int64)
        for b in blks:
            off_b[:, s, :, b] = run
            run += cnt[:, s, :, b]
        tot_sq[:, s, :] = run

    # compile-time layout. Each one-hot seg lives in one of two column
    # spaces: "dma" (host-expanded [P,P] tiles streamed from DRAM) or "dve"
    # (compact dstloc column expanded on-chip by the Vector engine). split
    # mode alternates segs to halve the DMA stream while keeping DVE under
    # its idle budget.
    instrs = []      # (s, q, nt, icol0, tg0)
    oh_segs = []     # (s, q, b, t_lo, t_hi, ohcol0, space)
    mm_by_block = {}  # b -> [(s, q, j, uid)]
    icol = 0
    tg = 0
    ohcols = {"dma": 0, "dve": 0}
    uid = 0
    seg_i = 0
    for s in range(NSC):
        for q in range(NQ):
            nt = max(1, int((tot_sq[:, s, q].max() + P - 1) // P))
            instrs.append((s, q, nt, icol, tg))
            for b in sblocks[s]:
                n_cb = cnt[:, s, q, b]
                if n_cb.max() == 0:
                    continue
                o0 = off_b[:, s, q, b]
                o1 = o0 + n_cb
                act = n_cb > 0
                t_lo = int((o0[act] // P).min())
                t_hi = int(((o1[act] - 1) // P).max())
                if OH_MODE == "dma":
                    space = "dma"
                elif OH_MODE == "dve":
                    space = "dve"
                else:
                    space = "dma" if seg_i % 2 == 0 else "dve"
                seg_i += 1
                oh_segs.append((s, q, b, t_lo, t_hi, ohcols[space], space, uid))
                lst = mm_by_block.setdefault(b, [])
                for j in range(t_lo, t_hi + 1):
                    lst.append((s, q, j, uid))
                    uid += 1
                ohcols[space] += t_hi - t_lo + 1
            icol += (nt * P) // 16
            tg += nt
    T_total = tg
    T_dma = max(1, ohcols["dma"])
    T_dve = max(1, ohcols["dve"])
    icols_total = icol
    maxnt = max(i[2] for i in instrs)
    instr_by_sq = {(s, q): (nt, icol0, tg0) for (s, q, nt, icol0, tg0) in instrs}

    # host aggregates for the colour/degree channel
    deg = np.bincount(dst, minlength=N).astype(np.float32)
    csum = np.bincount(dst, weights=cflat[src], minlength=N).astype(np.float32)

    in_maps = []
    for m in range(NC):
        sel = core == m
        sm = src[sel]
        dm = dst[sel]
        bm = (dm % NPC) // P
        d_loc = ((dm % NPC) % P).astype(np.float32)
        qm = sm // QROWS
        ls = (sm % QROWS).astype(np.int64)
        ssm = bm // GB
        key = (((ssm * NQ + qm) * NBLK + bm) * QROWS + ls)
        order = np.argsort(key, kind="stable")
        bm, d_loc, qm, ls, ssm = (bm[order], d_loc[order], qm[order],
                                  ls[order], ssm[order])

        # slot position = per-core offset of (s,q,b) + rank within group
        gid = ((ssm * NQ + qm) * NBLK + bm)
        gcnt = np.bincount(gid, minlength=NSC * NQ * NBLK)
        gstart = np.concatenate([[0], np.cumsum(gcnt)[:-1]])
        rank = np.arange(len(gid)) - gstart[gid]
        pos = off_b[m].reshape(-1)[gid] + rank        # slot within (s,q)

        # pad slots use idx 0 (a valid row, masked by zero rows in the
        # host-built one-hot). Trailing -1 + per-core num_idxs_reg truncation
        # works but measured slower and with higher variance than plain
        # full-length gathers.
        idx16 = np.zeros((16, icols_total), np.int16)
        for (s, q, nt, icol0, tg0) in instrs:
            sel_i = (ssm == s) & (qm == q)
            if not sel_i.any():
                continue
            n_ = pos[sel_i]
            idx16[n_ % 16, icol0 + n_ // 16] = ls[sel_i].astype(np.int16)
        ohh = np.zeros((P, T_dma * P), BF16)
        dstloc = np.full((P, T_dve), SENT, np.float32)
        for (s, q, b, t_lo, t_hi, ohcol0, space, uid0) in oh_segs:
            sel_i = (ssm == s) & (qm == q) & (bm == b)
            if not sel_i.any():
                continue
            p_ = pos[sel_i]
            if space == "dma":
                ohh[p_ % P, (ohcol0 + p_ // P - t_lo) * P
                    + d_loc[sel_i].astype(np.int64)] = 1.0
            else:
                dstloc[p_ % P, ohcol0 + p_ // P - t_lo] = d_loc[sel_i]
        idx16 = np.tile(idx16, (8, 1))
        nidx = np.array([[tot_sq[m, s, q] for (s, q, nt, icol0, tg0) in instrs]],
                        np.int32)

        xo = np.zeros((P, NPAD), np.float32)
        xo[:, :NPC] = x[m * NPC:(m + 1) * NPC].T
        zcd = np.zeros((2, NPAD), np.float32)
        zcd[0, :NPC] = (opeps * cflat[m * NPC:(m + 1) * NPC]
                        + csum[m * NPC:(m + 1) * NPC])
        zcd[1, :NPC] = opeps + deg[m * NPC:(m + 1) * NPC]
        zcd = zcd.astype(BF16)

        mp = {
            "xt": x.astype(BF16), "idx16": idx16,
            "xoT": xo, "zcd": zcd, "nidx": nidx,
        }
        if OH_MODE != "dve":
            mp["oh"] = ohh
        if OH_MODE != "dma":
            mp["dstloc"] = dstloc.astype(BF16)
        in_maps.append(mp)

    # shared weights
    W1 = np.asarray(W1, np.float32)
    w1a = np.ascontiguousarray(W1[:D])
    W1c = W1[D:]
    w1cd = np.ascontiguousarray(np.stack([
        np.asarray(colour_W, np.float32)[0] @ W1c,
        np.asarray(colour_b, np.float32) @ W1c,
    ]))
    consts = {
        "w1a": w1a.astype(BF16), "w1cd": w1cd.astype(BF16),
        "w2": np.ascontiguousarray(np.asarray(W2, np.float32)).astype(BF16),
        "b2t": np.tile(np.asarray(b2, np.float32).reshape(1, -1), (P, 1)),
        "gammab": np.stack([np.asarray(gamma, np.float32),
                            np.asarray(beta, np.float32)], axis=1),
    }
    if OH_MODE != "dma":
        consts["iota"] = np.tile(np.arange(P, dtype=np.float32),
                                 (P, 1)).astype(BF16)
    for mp in in_maps:
        mp.update(consts)

    meta = {
        "instrs": instrs, "instr_by_sq": instr_by_sq, "oh_segs": oh_segs,
        "mm_by_block": mm_by_block, "sblocks": sblocks,
        "T_total": T_total, "T_dma": T_dma, "T_dve": T_dve,
        "icols_total": icols_total, "maxnt": maxnt, "NSC": NSC,
    }
    dims = {"N": N, "D": D, "NPC": NPC, "NBLK": NBLK, "NPAD": NPAD,
            "QROWS": QROWS, "opeps": opeps}
    return in_maps, meta, dims


# ----------------------------------------------------------------- bass build

def _build(meta, dims, num_devices):
    import concourse.bacc as bacc
    import concourse.mybir as mybir
    import concourse.tile as tile

    N, D = dims["N"], dims["D"]
    NBLK, NPAD, QROWS = dims["NBLK"], dims["NPAD"], dims["QROWS"]
    opeps = dims["opeps"]
    T_dma = meta["T_dma"]
    T_dve = meta["T_dve"]
    icols_total = meta["icols_total"]
    maxnt = meta["maxnt"]
    f32 = mybir.dt.float32
    bf16 = mybir.dt.bfloat16

    nc = bacc.Bacc("TRN2", target_bir_lowering=False, debug=False,
                   num_devices=num_devices, num_swdge_queues=4)
    xt = nc.declare_dram_parameter("xt", [N, D], bf16, isOutput=False)
    idx16 = nc.declare_dram_parameter("idx16", [P, icols_total], mybir.dt.int16, isOutput=False)
    if OH_MODE != "dve":
        oh_d = nc.declare_dram_parameter("oh", [P, T_dma * P], bf16, isOutput=False)
    if OH_MODE != "dma":
        dstloc_d = nc.declare_dram_parameter("dstloc", [P, T_dve], bf16, isOutput=False)
    xoT_d = nc.declare_dram_parameter("xoT", [P, NPAD], f32, isOutput=False)
    zcd_d = nc.declare_dram_parameter("zcd", [2, NPAD], bf16, isOutput=False)
    w1a_d = nc.declare_dram_parameter("w1a", [D, P], bf16, isOutput=False)
    w1cd_d = nc.declare_dram_parameter("w1cd", [2, P], bf16, isOutput=False)
    w2_d = nc.declare_dram_parameter("w2", [P, P], bf16, isOutput=False)
    b2_d = nc.declare_dram_parameter("b2t", [P, P], f32, isOutput=False)
    gb_d = nc.declare_dram_parameter("gammab", [P, 2], f32, isOutput=False)
    if OH_MODE != "dma":
        iota_d = nc.declare_dram_parameter("iota", [P, P], bf16, isOutput=False)
    n_instr = len(meta["instrs"])
    nidx_d = nc.declare_dram_parameter("nidx", [1, n_instr], mybir.dt.int32, isOutput=False)
    out_d = nc.declare_dram_parameter("out", [NPAD, P], f32, isOutput=True)
    cc_in = nc.dram_tensor("cc_in", [P, 2], f32)
    cc_out = nc.dram_tensor("cc_out", [P, 2], f32, addr_space="Shared")

    add = mybir.AluOpType.add
    mult = mybir.AluOpType.mult
    eq = mybir.AluOpType.is_equal
    AF = mybir.ActivationFunctionType

    instr_by_sq = meta["instr_by_sq"]
    oh_segs = meta["oh_segs"]
    mm_by_block = meta["mm_by_block"]
    sblocks = meta["sblocks"]
    NSC = meta["NSC"]

    segs_by_sq = {}
    for (s, q, b, t_lo, t_hi, ohcol0, space, uid0) in oh_segs:
        segs_by_sq.setdefault((s, q), []).append(
            (b, t_lo, t_hi, ohcol0, space, uid0))
    # per-(s,q) contiguous dma-space column span (emission order s,q,b asc)
    sq_oh = {}
    for (s, q, b, t_lo, t_hi, ohcol0, space, uid0) in oh_segs:
        if space != "dma":
            continue
        base, tot = sq_oh.get((s, q), (None, 0))
        if base is None:
            base = ohcol0
        sq_oh[(s, q)] = (base, ohcol0 + (t_hi - t_lo + 1) - base)
    max_oh_sq = max((v[1] for v in sq_oh.values()), default=1)
    # superchunk icol range
    sc_icol = {}
    for (s, q), (nt, icol0, tg0) in sorted(instr_by_sq.items()):
        a, tot = sc_icol.get(s, (None, 0))
        if a is None:
            a = icol0
        sc_icol[s] = (a, tot + (nt * P) // 16)
    max_icols_sc = max(v[1] for v in sc_icol.values())

    with tile.TileContext(nc) as tc:
        with (
            tc.tile_pool(name="const", bufs=1) as cp,
            tc.tile_pool(name="gp", bufs=8) as gp,
            tc.tile_pool(name="ohp", bufs=4) as ohp,
            tc.tile_pool(name="dvp", bufs=12) as dvp,
            tc.tile_pool(name="ip", bufs=3) as ip,
            tc.tile_pool(name="xop", bufs=3) as xop,
            tc.tile_pool(name="zp", bufs=3) as zp,
            tc.tile_pool(name="y1p", bufs=1) as y1p,
            tc.tile_pool(name="scr", bufs=3) as scr,
            tc.tile_pool(name="stat", bufs=4) as stp,
            tc.tile_pool(name="psA", bufs=1, space="PSUM") as psA,
            tc.tile_pool(name="psC", bufs=2, space="PSUM") as psC,
        ):
            # ---- constants
            w1a_s = cp.tile([D, P], bf16)
            nc.sync.dma_start(w1a_s[:], w1a_d[:])
            w1cd_s = cp.tile([2, P], bf16)
            nc.sync.dma_start(w1cd_s[:], w1cd_d[:])
            w2_s = cp.tile([P, P], bf16)
            nc.sync.dma_start(w2_s[:], w2_d[:])
            b2_s = cp.tile([P, P], f32)
            nc.sync.dma_start(b2_s[:], b2_d[:])
            gb_s = cp.tile([P, 2], f32)
            nc.sync.dma_start(gb_s[:], gb_d[:])
            if OH_MODE != "dma":
                iota_s = cp.tile([P, P], bf16)
                nc.sync.dma_start(iota_s[:], iota_d[:])
                dstloc_s = cp.tile([P, T_dve], bf16)
                nc.sync.dma_start(dstloc_s[:], dstloc_d[:])
            nidx_s = cp.tile([1, n_instr], mybir.dt.int32)
            nc.sync.dma_start(nidx_s[:], nidx_d[:])
            acc1 = cp.tile([P, 1], f32)
            nc.vector.memset(acc1[:], 0.0)
            acc2 = cp.tile([P, 1], f32)
            nc.vector.memset(acc2[:], 0.0)
            zeros1 = cp.tile([P, 1], f32)
            nc.vector.memset(zeros1[:], 0.0)
            epsb = cp.tile([P, 1], f32)
            nc.vector.memset(epsb[:], 1e-5)
            y1store = y1p.tile([P, NPAD], f32)
            # zero the gather buffers once: truncated (trailing -1) gathers
            # leave tail tiles unwritten, and uninitialized SBUF could hold
            # NaN which survives the one-hot masking (NaN * 0 = NaN).
            for i in range(8):
                ginit = gp.tile([P, maxnt, P], bf16, tag="g", name=f"ginit{i}")
                nc.vector.memset(ginit[:], 0.0)

            # ---- phase 1
            for s in range(NSC):
                icol0_sc, icols_sc = sc_icol[s]
                it = ip.tile([P, max_icols_sc], mybir.dt.int16, tag="idx",
                             name=f"it_{s}")
                nc.sync.dma_start(it[:, :icols_sc],
                                  idx16[:, icol0_sc:icol0_sc + icols_sc])
                gbufs = {}
                ohbufs = {}
                px_tiles = {}
                first_mm = {b: mm_by_block[b][0] for b in sblocks[s]
                            if mm_by_block.get(b)}
                last_mm = {b: mm_by_block[b][-1] for b in sblocks[s]
                           if mm_by_block.get(b)}
                for q in range(NQ):
                    nt, icol0, tg0 = instr_by_sq[(s, q)]
                    icols = (nt * P) // 16
                    g = gp.tile([P, maxnt, P], bf16, tag="g", name=f"g_{s}_{q}")
                    if NO_GATHER:
                        nc.vector.memset(g[:, :1, :8], 0.0)
                    else:
                        nc.gpsimd.dma_gather(
                            out_ap=g[:, :nt, :],
                            in_ap=xt[q * QROWS:(q + 1) * QROWS, :],
                            idxs_ap=it[:, icol0 - icol0_sc:icol0 - icol0_sc + icols],
                            num_idxs=nt * P,
                            num_idxs_reg=nt * P,
                            elem_size=D,
                            single_packet=False,
                            queue_num=q,
                        )
                    gbufs[q] = g
                    if (s, q) in sq_oh:
                        base, ncols = sq_oh[(s, q)]
                        ohq = ohp.tile([P, max_oh_sq * P], bf16,
                                       tag="oh", name=f"oh_{s}_{q}")
                        nc.sync.dma_start(ohq[:, :ncols * P],
                                         oh_d[:, base * P:(base + ncols) * P])
                        ohbufs[q] = (ohq, base)
                # process SC blocks at a time: px PSUM tags cycle every SC
                # blocks, so a half's epilogues must retire before the next
                # half's matmuls reuse the banks (else PE-order deadlock).
                blks_s = sblocks[s]
                halves = [blks_s[i:i + SC] for i in range(0, len(blks_s), SC)]
                for half in halves:
                  for q in range(NQ):
                    for (b, t_lo, t_hi, ohcol0, space, uid0) in \
                            segs_by_sq.get((s, q), []):
                        if b not in half:
                            continue
                        if b not in px_tiles:
                            px_tiles[b] = psA.tile([P, P], f32,
                                                   tag=f"px{b % SC}",
                                                   name=f"px_{b}")
                        px_t = px_tiles[b]
                        g = gbufs[q]
                        ntb = t_hi - t_lo + 1
                        if space == "dma":
                            ohq, base = ohbufs[q]
                            oh_ap = ohq[:, (ohcol0 - base) * P:
                                        (ohcol0 - base + ntb) * P]
                        else:
                            oh = dvp.tile([P, ntb * P], bf16, tag="ohv",
                                          name=f"oh_{s}_{q}_{b}")
                            nc.vector.tensor_tensor(
                                out=oh[:].rearrange("p (t c) -> p t c", c=P),
                                in0=dstloc_s[:, ohcol0:ohcol0 + ntb]
                                    .rearrange("p (t u) -> p t u", u=1)
                                    .to_broadcast([P, ntb, P]),
                                in1=iota_s[:].rearrange("p (u c) -> p u c", u=1)
                                    .to_broadcast([P, ntb, P]),
                                op=eq,
                            )
                            oh_ap = oh[:]
                        for j in range(t_lo, t_hi + 1):
                            jo = j - t_lo
                            nc.tensor.matmul(
                                out=px_t[:],
                                lhsT=g[:, j, :],
                                rhs=oh_ap[:, jo * P:(jo + 1) * P],
                                start=((s, q, j, uid0 + jo) == first_mm[b]),
                                stop=((s, q, j, uid0 + jo) == last_mm[b]),
                            )
                  # per-half epilogues (emission order defines psum acc)
                  for b in half:
                    mms = mm_by_block.get(b, [])
                    xo = xop.tile([P, P], f32, tag="xo")
                    nc.scalar.dma_start(xo[:], xoT_d[:, b * P:(b + 1) * P])
                    zcd_t = xop.tile([2, P], bf16, tag="zcd")
                    nc.scalar.dma_start(zcd_t[:], zcd_d[:, b * P:(b + 1) * P])
                    zx = zp.tile([P, P], bf16, tag="zx")
                    if mms:
                        px_t = px_tiles[b]
                        nc.vector.scalar_tensor_tensor(
                            out=zx[:], in0=xo[:], scalar=opeps, in1=px_t[:],
                            op0=mult, op1=add)
                    else:
                        nc.scalar.activation(out=zx[:], in_=xo[:],
                                             func=AF.Copy, scale=opeps)
                    py = psC.tile([P, P], f32, tag="py1")
                    nc.tensor.matmul(out=py[:], lhsT=w1a_s[:], rhs=zx[:],
                                     start=True, stop=False)
                    nc.tensor.matmul(out=py[:], lhsT=w1cd_s[:], rhs=zcd_t[:],
                                     start=False, stop=True)
                    r1 = stp.tile([P, 1], f32, tag="r1")
                    nc.scalar.activation(out=y1store[:, b * P:(b + 1) * P],
                                         in_=py[:], func=AF.Copy, accum_out=r1[:])
                    sqt = scr.tile([P, P], f32, tag="sq")
                    r2 = stp.tile([P, 1], f32, tag="r2")
                    nc.scalar.activation(out=sqt[:], in_=py[:], func=AF.Square,
                                         bias=zeros1[:], accum_out=r2[:])
                    nc.vector.tensor_add(acc1[:], acc1[:], r1[:])
                    nc.vector.tensor_add(acc2[:], acc2[:], r2[:])

            # ---- phase 2: BN stats allreduce
            st = stp.tile([P, 2], f32, tag="st")
            nc.vector.tensor_copy(st[:, 0:1], acc1[:])
            nc.vector.tensor_copy(st[:, 1:2], acc2[:])
            nc.sync.dma_start(cc_in[:], st[:])
            nc.gpsimd.collective_compute(
                "AllReduce", add,
                replica_groups=[list(range(num_devices))],
                ins=[cc_in[:]], outs=[cc_out[:]],
            )
            red = stp.tile([P, 2], f32, tag="red")
            nc.sync.dma_start(red[:], cc_out[:])
            mu = stp.tile([P, 1], f32, tag="mu")
            nc.scalar.activation(out=mu[:], in_=red[:, 0:1], func=AF.Copy,
                                 scale=1.0 / N)
            m2 = stp.tile([P, 1], f32, tag="m2")
            nc.scalar.activation(out=m2[:], in_=red[:, 1:2], func=AF.Copy,
                                 scale=1.0 / N)
            var = stp.tile([P, 1], f32, tag="var")
            negmu = stp.tile([P, 1], f32, tag="negmu")
            nc.scalar.activation(out=negmu[:], in_=mu[:], func=AF.Copy, scale=-1.0)
            nc.vector.scalar_tensor_tensor(out=var[:], in0=mu[:], scalar=negmu[:],
                                           in1=m2[:], op0=mult, op1=add)
            sd = stp.tile([P, 1], f32, tag="sd")
            nc.scalar.activation(out=sd[:], in_=var[:], func=AF.Sqrt, bias=epsb[:])
            inv = stp.tile([P, 1], f32, tag="inv")
            nc.vector.reciprocal(inv[:], sd[:])
            a_s = stp.tile([P, 1], f32, tag="a_s")
            nc.vector.tensor_mul(a_s[:], inv[:], gb_s[:, 0:1])
            nmua = stp.tile([P, 1], f32, tag="nmua")
            nc.scalar.activation(out=nmua[:], in_=a_s[:], func=AF.Copy, scale=-1.0)
            bb_t = stp.tile([P, 1], f32, tag="bb")
            nc.vector.scalar_tensor_tensor(out=bb_t[:], in0=mu[:], scalar=nmua[:],
                                           in1=gb_s[:, 1:2], op0=mult, op1=add)

            # ---- phase 3: BN+ReLU, second linear, output
            for b in range(NBLK):
                rt = scr.tile([P, P], bf16, tag="rt")
                nc.scalar.activation(out=rt[:], in_=y1store[:, b * P:(b + 1) * P],
                                     func=AF.Relu, bias=bb_t[:], scale=a_s[:])
                py2 = psA.tile([P, P], f32, tag="px0")
                nc.tensor.matmul(out=py2[:], lhsT=rt[:], rhs=w2_s[:],
                                 start=True, stop=True)
                ot = scr.tile([P, P], f32, tag="ot")
                nc.vector.tensor_tensor(out=ot[:], in0=py2[:], in1=b2_s[:],
                                        op=add)
                nc.scalar.dma_start(out_d[b * P:(b + 1) * P, :], ot[:])

    nc.finalize()
    return nc


# ----------------------------------------------------------------- entry

_CACHE = {}


def kernel(**inputs):
    from concourse.bass_utils import run_bass_kernel_spmd

    in_maps, meta, dims = _prep(
        inputs["x"], inputs["c"], inputs["edge_index"], inputs["colour_W"],
        inputs["colour_b"], inputs["eps"], inputs["W1"], inputs["gamma"],
        inputs["beta"], inputs["W2"], inputs["b2"])

    key = (dims["N"], dims["D"], meta["T_total"], meta["T_dma"],
           meta["T_dve"], dims["opeps"])
    if key not in _CACHE:
        _CACHE[key] = _build(meta, dims, NC)
    nc = _CACHE[key]

    res = run_bass_kernel_spmd(nc, in_maps, list(range(NC)))
    NPC = dims["NPC"]
    out = np.empty((dims["N"], P), np.float32)
    for m in range(NC):
        out[m * NPC:(m + 1) * NPC] = res.results[m]["out"][:NPC]
    return out



# revision 59
# speedup vs baseline: 2.5751x; 2.5751x over previous
"""ColourCatGINConv on 8 TRN2 NeuronCores.

Strategy: shard GIN aggregation by destination-node range (12500 nodes/core).
Each core gathers x[src] rows for its incoming edges via SWDGE dma_gather
(int16 indices -> the 100k-row table is split into 4 quadrants of 25000 rows),
and segment-sums them with a one-hot matmul on the TensorEngine:

    P_x[f, d] = sum_e x[src_e][f] * onehot[e, d]     (lhsT = gathered rows)

Gather slots are packed contiguously per (superchunk, quadrant) instruction
per core — no interior padding. Tiles may straddle dst blocks and block
boundaries differ per core; each block's one-hot covers the compile-time
UNION tile range over all cores, with foreign/pad slots masked to a sentinel
in the per-core dstloc table. Trailing pad indices are negative so the SWDGE
ucode truncates them before descriptor generation (the Q7 descriptor rate,
~8ns/idx, is the kernel bottleneck).

The colour channel never touches the device gather: with G = [x, c, 1],
y1 = Z @ [[W1_x], [colour_W @ W1_c], [colour_b @ W1_c]] and the [c, 1]
aggregates (per-dst colour sum and degree) are index metadata computed on
host and shipped as a z-ready [2, NPAD] input. BN batch stats are
partial-summed per core and AllReduced.
"""

import os
import numpy as np
import ml_dtypes

NO_GATHER = bool(int(os.environ.get("KERNEL_NO_GATHER", "0")))
OH_MODE = os.environ.get("KERNEL_OH", "dma")   # dma | split
BF16 = ml_dtypes.bfloat16
FP8 = ml_dtypes.float8_e4m3
P = 128
NC = 8
NQ = 4
SC = 6          # dst blocks (128 nodes each) per superchunk
SENT = 255.0    # dstloc sentinel for foreign/pad slots


# ----------------------------------------------------------------- host prep

def _prep(x, c, edge_index, colour_W, colour_b, eps, W1, gamma, beta, W2, b2):
    N, D = x.shape
    NPC = N // NC
    NBLK = (NPC + P - 1) // P
    NPAD = NBLK * P
    QROWS = N // NQ
    NSC = (NBLK + SC - 1) // SC
    opeps = float(1.0 + np.asarray(eps, np.float32).reshape(-1)[0])

    x = np.ascontiguousarray(x, np.float32)
    cflat = np.asarray(c, np.float32).reshape(-1)
    src = np.asarray(edge_index[0], np.int64)
    dst = np.asarray(edge_index[1], np.int64)

    core = dst // NPC
    b_all = (dst % NPC) // P
    q_all = src // QROWS
    # gather instructions span GSC superchunks (coarser granularity cuts the
    # per-instruction ceil padding and fixed cost); PSUM epilogues still
    # process SC blocks at a time.
    GSC = 1
    GB = GSC * SC
    NSP = (NBLK + GB - 1) // GB
    s_all = b_all // GB
    sblocks = [list(range(s * GB, min((s + 1) * GB, NBLK))) for s in range(NSP)]
    NSC = NSP

    # per-core per-(s,q,b) counts -> cumulative offsets within each (s,q)
    grp_b = ((core * NSC + s_all) * NQ + q_all) * NBLK + b_all
    cnt = np.bincount(grp_b, minlength=NC * NSC * NQ * NBLK)
    cnt = cnt.reshape(NC, NSC, NQ, NBLK)

    # per-core block start offsets within (s,q); per-core (s,q) totals
    off_b = np.zeros_like(cnt)
    tot_sq = np.zeros((NC, NSC, NQ), np.int64)
    for s in range(NSC):
        blks = sblocks[s]
        run = np.zeros((NC, NQ), np.int64)
        for b in blks:
            off_b[:, s, :, b] = run
            run += cnt[:, s, :, b]
        tot_sq[:, s, :] = run

    # compile-time layout. Each one-hot seg lives in one of two column
    # spaces: "dma" (host-expanded [P,P] tiles streamed from DRAM) or "dve"
    # (compact dstloc column expanded on-chip by the Vector engine). split
    # mode alternates segs to halve the DMA stream while keeping DVE under
    # its idle budget.
    instrs = []      # (s, q, nt, icol0, tg0)
    oh_segs = []     # (s, q, b, t_lo, t_hi, ohcol0, space)
    mm_by_block = {}  # b -> [(s, q, j, uid)]
    icol = 0
    tg = 0
    ohcols = {"dma": 0, "dve": 0}
    uid = 0
    seg_i = 0
    for s in range(NSC):
        for q in range(NQ):
            nt = max(1, int((tot_sq[:, s, q].max() + P - 1) // P))
            instrs.append((s, q, nt, icol, tg))
            for b in sblocks[s]:
                n_cb = cnt[:, s, q, b]
                if n_cb.max() == 0:
                    continue
                o0 = off_b[:, s, q, b]
                o1 = o0 + n_cb
                act = n_cb > 0
                t_lo = int((o0[act] // P).min())
                t_hi = int(((o1[act] - 1) // P).max())
                if OH_MODE == "dma":
                    space = "dma"
                elif OH_MODE == "dve":
                    space = "dve"
                else:
                    space = "dma" if seg_i % 2 == 0 else "dve"
                seg_i += 1
                oh_segs.append((s, q, b, t_lo, t_hi, ohcols[space], space, uid))
                lst = mm_by_block.setdefault(b, [])
                for j in range(t_lo, t_hi + 1):
                    lst.append((s, q, j, uid))
                    uid += 1
                ohcols[space] += t_hi - t_lo + 1
            icol += (nt * P) // 16
            tg += nt
    T_total = tg
    T_dma = max(1, ohcols["dma"])
    T_dve = max(1, ohcols["dve"])
    icols_total = icol
    maxnt = max(i[2] for i in instrs)
    instr_by_sq = {(s, q): (nt, icol0, tg0) for (s, q, nt, icol0, tg0) in instrs}

    # host aggregates for the colour/degree channel
    deg = np.bincount(dst, minlength=N).astype(np.float32)
    csum = np.bincount(dst, weights=cflat[src], minlength=N).astype(np.float32)

    in_maps = []
    for m in range(NC):
        sel = core == m
        sm = src[sel]
        dm = dst[sel]
        bm = (dm % NPC) // P
        d_loc = ((dm % NPC) % P).astype(np.float32)
        qm = sm // QROWS
        ls = (sm % QROWS).astype(np.int64)
        ssm = bm // GB
        key = (((ssm * NQ + qm) * NBLK + bm) * QROWS + ls)
        order = np.argsort(key, kind="stable")
        bm, d_loc, qm, ls, ssm = (bm[order], d_loc[order], qm[order],
                                  ls[order], ssm[order])

        # slot position = per-core offset of (s,q,b) + rank within group
        gid = ((ssm * NQ + qm) * NBLK + bm)
        gcnt = np.bincount(gid, minlength=NSC * NQ * NBLK)
        gstart = np.concatenate([[0], np.cumsum(gcnt)[:-1]])
        rank = np.arange(len(gid)) - gstart[gid]
        pos = off_b[m].reshape(-1)[gid] + rank        # slot within (s,q)

        # pad slots use idx 0 (a valid row, masked by zero rows in the
        # host-built one-hot). Trailing -1 + per-core num_idxs_reg truncation
        # works but measured slower and with higher variance than plain
        # full-length gathers.
        idx16 = np.zeros((16, icols_total), np.int16)
        for (s, q, nt, icol0, tg0) in instrs:
            sel_i = (ssm == s) & (qm == q)
            if not sel_i.any():
                continue
            n_ = pos[sel_i]
            idx16[n_ % 16, icol0 + n_ // 16] = ls[sel_i].astype(np.int16)
        ohh = np.zeros((P, T_dma * P), BF16)
        dstloc = np.full((P, T_dve), SENT, np.float32)
        for (s, q, b, t_lo, t_hi, ohcol0, space, uid0) in oh_segs:
            sel_i = (ssm == s) & (qm == q) & (bm == b)
            if not sel_i.any():
                continue
            p_ = pos[sel_i]
            if space == "dma":
                ohh[p_ % P, (ohcol0 + p_ // P - t_lo) * P
                    + d_loc[sel_i].astype(np.int64)] = 1.0
            else:
                dstloc[p_ % P, ohcol0 + p_ // P - t_lo] = d_loc[sel_i]
        idx16 = np.tile(idx16, (8, 1))
        nidx = np.array([[tot_sq[m, s, q] for (s, q, nt, icol0, tg0) in instrs]],
                        np.int32)

        xo = np.zeros((P, NPAD), np.float32)
        xo[:, :NPC] = x[m * NPC:(m + 1) * NPC].T
        zcd = np.zeros((2, NPAD), np.float32)
        zcd[0, :NPC] = (opeps * cflat[m * NPC:(m + 1) * NPC]
                        + csum[m * NPC:(m + 1) * NPC])
        zcd[1, :NPC] = opeps + deg[m * NPC:(m + 1) * NPC]
        zcd = zcd.astype(BF16)

        mp = {
            "xt": x.astype(BF16), "idx16": idx16,
            "xoT": xo, "zcd": zcd, "nidx": nidx,
        }
        if OH_MODE != "dve":
            mp["oh"] = ohh
        if OH_MODE != "dma":
            mp["dstloc"] = dstloc.astype(BF16)
        in_maps.append(mp)

    # shared weights
    W1 = np.asarray(W1, np.float32)
    w1a = np.ascontiguousarray(W1[:D])
    W1c = W1[D:]
    w1cd = np.ascontiguousarray(np.stack([
        np.asarray(colour_W, np.float32)[0] @ W1c,
        np.asarray(colour_b, np.float32) @ W1c,
    ]))
    consts = {
        "w1a": w1a.astype(BF16), "w1cd": w1cd.astype(BF16),
        "w2": np.ascontiguousarray(np.asarray(W2, np.float32)).astype(BF16),
        "b2t": np.tile(np.asarray(b2, np.float32).reshape(1, -1), (P, 1)),
        "gammab": np.stack([np.asarray(gamma, np.float32),
                            np.asarray(beta, np.float32)], axis=1),
    }
    if OH_MODE != "dma":
        consts["iota"] = np.tile(np.arange(P, dtype=np.float32),
                                 (P, 1)).astype(BF16)
    for mp in in_maps:
        mp.update(consts)

    meta = {
        "instrs": instrs, "instr_by_sq": instr_by_sq, "oh_segs": oh_segs,
        "mm_by_block": mm_by_block, "sblocks": sblocks,
        "T_total": T_total, "T_dma": T_dma, "T_dve": T_dve,
        "icols_total": icols_total, "maxnt": maxnt, "NSC": NSC,
    }
    dims = {"N": N, "D": D, "NPC": NPC, "NBLK": NBLK, "NPAD": NPAD,
            "QROWS": QROWS, "opeps": opeps}
    return in_maps, meta, dims


# ----------------------------------------------------------------- bass build

def _build(meta, dims, num_devices):
    import concourse.bacc as bacc
    import concourse.mybir as mybir
    import concourse.tile as tile

    N, D = dims["N"], dims["D"]
    NBLK, NPAD, QROWS = dims["NBLK"], dims["NPAD"], dims["QROWS"]
    opeps = dims["opeps"]
    T_dma = meta["T_dma"]
    T_dve = meta["T_dve"]
    icols_total = meta["icols_total"]
    maxnt = meta["maxnt"]
    f32 = mybir.dt.float32
    bf16 = mybir.dt.bfloat16

    nc = bacc.Bacc("TRN2", target_bir_lowering=False, debug=False,
                   num_devices=num_devices, num_swdge_queues=4)
    xt = nc.declare_dram_parameter("xt", [N, D], bf16, isOutput=False)
    idx16 = nc.declare_dram_parameter("idx16", [P, icols_total], mybir.dt.int16, isOutput=False)
    if OH_MODE != "dve":
        oh_d = nc.declare_dram_parameter("oh", [P, T_dma * P], bf16, isOutput=False)
    if OH_MODE != "dma":
        dstloc_d = nc.declare_dram_parameter("dstloc", [P, T_dve], bf16, isOutput=False)
    xoT_d = nc.declare_dram_parameter("xoT", [P, NPAD], f32, isOutput=False)
    zcd_d = nc.declare_dram_parameter("zcd", [2, NPAD], bf16, isOutput=False)
    w1a_d = nc.declare_dram_parameter("w1a", [D, P], bf16, isOutput=False)
    w1cd_d = nc.declare_dram_parameter("w1cd", [2, P], bf16, isOutput=False)
    w2_d = nc.declare_dram_parameter("w2", [P, P], bf16, isOutput=False)
    b2_d = nc.declare_dram_parameter("b2t", [P, P], f32, isOutput=False)
    gb_d = nc.declare_dram_parameter("gammab", [P, 2], f32, isOutput=False)
    if OH_MODE != "dma":
        iota_d = nc.declare_dram_parameter("iota", [P, P], bf16, isOutput=False)
    n_instr = len(meta["instrs"])
    nidx_d = nc.declare_dram_parameter("nidx", [1, n_instr], mybir.dt.int32, isOutput=False)
    out_d = nc.declare_dram_parameter("out", [NPAD, P], f32, isOutput=True)
    cc_in = nc.dram_tensor("cc_in", [P, 2], f32)
    cc_out = nc.dram_tensor("cc_out", [P, 2], f32, addr_space="Shared")

    add = mybir.AluOpType.add
    mult = mybir.AluOpType.mult
    eq = mybir.AluOpType.is_equal
    AF = mybir.ActivationFunctionType

    instr_by_sq = meta["instr_by_sq"]
    oh_segs = meta["oh_segs"]
    mm_by_block = meta["mm_by_block"]
    sblocks = meta["sblocks"]
    NSC = meta["NSC"]

    GBP = max(len(bl) for bl in sblocks) * P
    segs_by_sq = {}
    for (s, q, b, t_lo, t_hi, ohcol0, space, uid0) in oh_segs:
        segs_by_sq.setdefault((s, q), []).append(
            (b, t_lo, t_hi, ohcol0, space, uid0))
    # per-(s,q) contiguous dma-space column span (emission order s,q,b asc)
    sq_oh = {}
    for (s, q, b, t_lo, t_hi, ohcol0, space, uid0) in oh_segs:
        if space != "dma":
            continue
        base, tot = sq_oh.get((s, q), (None, 0))
        if base is None:
            base = ohcol0
        sq_oh[(s, q)] = (base, ohcol0 + (t_hi - t_lo + 1) - base)
    max_oh_sq = max((v[1] for v in sq_oh.values()), default=1)
    # superchunk icol range
    sc_icol = {}
    for (s, q), (nt, icol0, tg0) in sorted(instr_by_sq.items()):
        a, tot = sc_icol.get(s, (None, 0))
        if a is None:
            a = icol0
        sc_icol[s] = (a, tot + (nt * P) // 16)
    max_icols_sc = max(v[1] for v in sc_icol.values())

    with tile.TileContext(nc) as tc:
        with (
            tc.tile_pool(name="const", bufs=1) as cp,
            tc.tile_pool(name="gp", bufs=10) as gp,
            tc.tile_pool(name="ohp", bufs=4) as ohp,
            tc.tile_pool(name="dvp", bufs=12) as dvp,
            tc.tile_pool(name="ip", bufs=3) as ip,
            tc.tile_pool(name="xop", bufs=3) as xop,
            tc.tile_pool(name="zp", bufs=3) as zp,
            tc.tile_pool(name="y1p", bufs=1) as y1p,
            tc.tile_pool(name="scr", bufs=3) as scr,
            tc.tile_pool(name="stat", bufs=4) as stp,
            tc.tile_pool(name="psA", bufs=1, space="PSUM") as psA,
            tc.tile_pool(name="psC", bufs=2, space="PSUM") as psC,
        ):
            # ---- constants
            w1a_s = cp.tile([D, P], bf16)
            nc.sync.dma_start(w1a_s[:], w1a_d[:])
            w1cd_s = cp.tile([2, P], bf16)
            nc.sync.dma_start(w1cd_s[:], w1cd_d[:])
            w2_s = cp.tile([P, P], bf16)
            nc.sync.dma_start(w2_s[:], w2_d[:])
            b2_s = cp.tile([P, P], f32)
            nc.sync.dma_start(b2_s[:], b2_d[:])
            gb_s = cp.tile([P, 2], f32)
            nc.sync.dma_start(gb_s[:], gb_d[:])
            if OH_MODE != "dma":
                iota_s = cp.tile([P, P], bf16)
                nc.sync.dma_start(iota_s[:], iota_d[:])
                dstloc_s = cp.tile([P, T_dve], bf16)
                nc.sync.dma_start(dstloc_s[:], dstloc_d[:])
            nidx_s = cp.tile([1, n_instr], mybir.dt.int32)
            nc.sync.dma_start(nidx_s[:], nidx_d[:])
            acc1 = cp.tile([P, 1], f32)
            nc.vector.memset(acc1[:], 0.0)
            acc2 = cp.tile([P, 1], f32)
            nc.vector.memset(acc2[:], 0.0)
            zeros1 = cp.tile([P, 1], f32)
            nc.vector.memset(zeros1[:], 0.0)
            epsb = cp.tile([P, 1], f32)
            nc.vector.memset(epsb[:], 1e-5)
            y1store = y1p.tile([P, NPAD], f32)
            # zero the gather buffers once: truncated (trailing -1) gathers
            # leave tail tiles unwritten, and uninitialized SBUF could hold
            # NaN which survives the one-hot masking (NaN * 0 = NaN).
            for i in range(10):
                ginit = gp.tile([P, maxnt, P], bf16, tag="g", name=f"ginit{i}")
                nc.vector.memset(ginit[:], 0.0)

            # ---- phase 1
            for s in range(NSC):
                icol0_sc, icols_sc = sc_icol[s]
                it = ip.tile([P, max_icols_sc], mybir.dt.int16, tag="idx",
                             name=f"it_{s}")
                nc.sync.dma_start(it[:, :icols_sc],
                                  idx16[:, icol0_sc:icol0_sc + icols_sc])
                gbufs = {}
                ohbufs = {}
                px_tiles = {}
                first_mm = {b: mm_by_block[b][0] for b in sblocks[s]
                            if mm_by_block.get(b)}
                last_mm = {b: mm_by_block[b][-1] for b in sblocks[s]
                           if mm_by_block.get(b)}
                for q in range(NQ):
                    nt, icol0, tg0 = instr_by_sq[(s, q)]
                    icols = (nt * P) // 16
                    g = gp.tile([P, maxnt, P], bf16, tag="g", name=f"g_{s}_{q}")
                    if NO_GATHER:
                        nc.vector.memset(g[:, :1, :8], 0.0)
                    else:
                        nc.gpsimd.dma_gather(
                            out_ap=g[:, :nt, :],
                            in_ap=xt[q * QROWS:(q + 1) * QROWS, :],
                            idxs_ap=it[:, icol0 - icol0_sc:icol0 - icol0_sc + icols],
                            num_idxs=nt * P,
                            num_idxs_reg=nt * P,
                            elem_size=D,
                            single_packet=False,
                            queue_num=q,
                        )
                    gbufs[q] = g
                    if (s, q) in sq_oh:
                        base, ncols = sq_oh[(s, q)]
                        ohq = ohp.tile([P, max_oh_sq * P], bf16,
                                       tag="oh", name=f"oh_{s}_{q}")
                        nc.sync.dma_start(ohq[:, :ncols * P],
                                         oh_d[:, base * P:(base + ncols) * P])
                        ohbufs[q] = (ohq, base)
                # prefetch the whole superchunk's epilogue inputs in one DMA
                blks_s = sblocks[s]
                nb = len(blks_s)
                b0_sc = blks_s[0]
                xo_sc = xop.tile([P, GBP], f32, tag="xo")
                nc.scalar.dma_start(xo_sc[:, :nb * P],
                                    xoT_d[:, b0_sc * P:(b0_sc + nb) * P])
                zcd_sc = xop.tile([2, GBP], bf16, tag="zcd")
                nc.scalar.dma_start(zcd_sc[:, :nb * P],
                                    zcd_d[:, b0_sc * P:(b0_sc + nb) * P])
                # process SC blocks at a time: px PSUM tags cycle every SC
                # blocks, so a half's epilogues must retire before the next
                # half's matmuls reuse the banks (else PE-order deadlock).
                halves = [blks_s[i:i + SC] for i in range(0, len(blks_s), SC)]
                for half in halves:
                  for q in range(NQ):
                    for (b, t_lo, t_hi, ohcol0, space, uid0) in \
                            segs_by_sq.get((s, q), []):
                        if b not in half:
                            continue
                        if b not in px_tiles:
                            px_tiles[b] = psA.tile([P, P], f32,
                                                   tag=f"px{b % SC}",
                                                   name=f"px_{b}")
                        px_t = px_tiles[b]
                        g = gbufs[q]
                        ntb = t_hi - t_lo + 1
                        if space == "dma":
                            ohq, base = ohbufs[q]
                            oh_ap = ohq[:, (ohcol0 - base) * P:
                                        (ohcol0 - base + ntb) * P]
                        else:
                            oh = dvp.tile([P, ntb * P], bf16, tag="ohv",
                                          name=f"oh_{s}_{q}_{b}")
                            nc.vector.tensor_tensor(
                                out=oh[:].rearrange("p (t c) -> p t c", c=P),
                                in0=dstloc_s[:, ohcol0:ohcol0 + ntb]
                                    .rearrange("p (t u) -> p t u", u=1)
                                    .to_broadcast([P, ntb, P]),
                                in1=iota_s[:].rearrange("p (u c) -> p u c", u=1)
                                    .to_broadcast([P, ntb, P]),
                                op=eq,
                            )
                            oh_ap = oh[:]
                        for j in range(t_lo, t_hi + 1):
                            jo = j - t_lo
                            nc.tensor.matmul(
                                out=px_t[:],
                                lhsT=g[:, j, :],
                                rhs=oh_ap[:, jo * P:(jo + 1) * P],
                                start=((s, q, j, uid0 + jo) == first_mm[b]),
                                stop=((s, q, j, uid0 + jo) == last_mm[b]),
                            )
                  # per-half epilogues (emission order defines psum acc)
                  for b in half:
                    mms = mm_by_block.get(b, [])
                    bo = (b - b0_sc) * P
                    xo = xo_sc[:, bo:bo + P]
                    zcd_t = zcd_sc[:, bo:bo + P]
                    zx = zp.tile([P, P], bf16, tag="zx")
                    if mms:
                        px_t = px_tiles[b]
                        nc.vector.scalar_tensor_tensor(
                            out=zx[:], in0=xo, scalar=opeps, in1=px_t[:],
                            op0=mult, op1=add)
                    else:
                        nc.scalar.activation(out=zx[:], in_=xo,
                                             func=AF.Copy, scale=opeps)
                    py = psC.tile([P, P], f32, tag="py1")
                    nc.tensor.matmul(out=py[:], lhsT=w1a_s[:], rhs=zx[:],
                                     start=True, stop=False)
                    nc.tensor.matmul(out=py[:], lhsT=w1cd_s[:], rhs=zcd_t,
                                     start=False, stop=True)
                    r1 = stp.tile([P, 1], f32, tag="r1")
                    nc.scalar.activation(out=y1store[:, b * P:(b + 1) * P],
                                         in_=py[:], func=AF.Copy, accum_out=r1[:])
                    sqt = scr.tile([P, P], f32, tag="sq")
                    r2 = stp.tile([P, 1], f32, tag="r2")
                    nc.scalar.activation(out=sqt[:], in_=py[:], func=AF.Square,
                                         bias=zeros1[:], accum_out=r2[:])
                    nc.vector.tensor_add(acc1[:], acc1[:], r1[:])
                    nc.vector.tensor_add(acc2[:], acc2[:], r2[:])

            # ---- phase 2: BN stats allreduce
            st = stp.tile([P, 2], f32, tag="st")
            nc.vector.tensor_copy(st[:, 0:1], acc1[:])
            nc.vector.tensor_copy(st[:, 1:2], acc2[:])
            nc.sync.dma_start(cc_in[:], st[:])
            nc.gpsimd.collective_compute(
                "AllReduce", add,
                replica_groups=[list(range(num_devices))],
                ins=[cc_in[:]], outs=[cc_out[:]],
            )
            red = stp.tile([P, 2], f32, tag="red")
            nc.sync.dma_start(red[:], cc_out[:])
            mu = stp.tile([P, 1], f32, tag="mu")
            nc.scalar.activation(out=mu[:], in_=red[:, 0:1], func=AF.Copy,
                                 scale=1.0 / N)
            m2 = stp.tile([P, 1], f32, tag="m2")
            nc.scalar.activation(out=m2[:], in_=red[:, 1:2], func=AF.Copy,
                                 scale=1.0 / N)
            var = stp.tile([P, 1], f32, tag="var")
            negmu = stp.tile([P, 1], f32, tag="negmu")
            nc.scalar.activation(out=negmu[:], in_=mu[:], func=AF.Copy, scale=-1.0)
            nc.vector.scalar_tensor_tensor(out=var[:], in0=mu[:], scalar=negmu[:],
                                           in1=m2[:], op0=mult, op1=add)
            sd = stp.tile([P, 1], f32, tag="sd")
            nc.scalar.activation(out=sd[:], in_=var[:], func=AF.Sqrt, bias=epsb[:])
            inv = stp.tile([P, 1], f32, tag="inv")
            nc.vector.reciprocal(inv[:], sd[:])
            a_s = stp.tile([P, 1], f32, tag="a_s")
            nc.vector.tensor_mul(a_s[:], inv[:], gb_s[:, 0:1])
            nmua = stp.tile([P, 1], f32, tag="nmua")
            nc.scalar.activation(out=nmua[:], in_=a_s[:], func=AF.Copy, scale=-1.0)
            bb_t = stp.tile([P, 1], f32, tag="bb")
            nc.vector.scalar_tensor_tensor(out=bb_t[:], in0=mu[:], scalar=nmua[:],
                                           in1=gb_s[:, 1:2], op0=mult, op1=add)

            # ---- phase 3: BN+ReLU, second linear, output
            for b in range(NBLK):
                rt = scr.tile([P, P], bf16, tag="rt")
                nc.scalar.activation(out=rt[:], in_=y1store[:, b * P:(b + 1) * P],
                                     func=AF.Relu, bias=bb_t[:], scale=a_s[:])
                py2 = psA.tile([P, P], f32, tag="px0")
                nc.tensor.matmul(out=py2[:], lhsT=rt[:], rhs=w2_s[:],
                                 start=True, stop=True)
                ot = scr.tile([P, P], f32, tag="ot")
                nc.vector.tensor_tensor(out=ot[:], in0=py2[:], in1=b2_s[:],
                                        op=add)
                nc.scalar.dma_start(out_d[b * P:(b + 1) * P, :], ot[:])

    nc.finalize()
    return nc


# ----------------------------------------------------------------- entry

_CACHE = {}


def kernel(**inputs):
    from concourse.bass_utils import run_bass_kernel_spmd

    in_maps, meta, dims = _prep(
        inputs["x"], inputs["c"], inputs["edge_index"], inputs["colour_W"],
        inputs["colour_b"], inputs["eps"], inputs["W1"], inputs["gamma"],
        inputs["beta"], inputs["W2"], inputs["b2"])

    key = (dims["N"], dims["D"], meta["T_total"], meta["T_dma"],
           meta["T_dve"], dims["opeps"])
    if key not in _CACHE:
        _CACHE[key] = _build(meta, dims, NC)
    nc = _CACHE[key]

    res = run_bass_kernel_spmd(nc, in_maps, list(range(NC)))
    NPC = dims["NPC"]
    out = np.empty((dims["N"], P), np.float32)
    for m in range(NC):
        out[m * NPC:(m + 1) * NPC] = res.results[m]["out"][:NPC]
    return out

